# revision 35
# baseline (speedup 1.0000x reference)
"""Distributed CAP-memory loss kernel for 8 TRN2 NeuronCores (fp8 v2).

Problem (see reference): given unit-norm features [B=256, D=2048] and a
memory bank [6, 2000, 2048], compute
  loss = sum_cam mean_cam(per-camera proxy CE)
       + 0.5 * sum_cam mean_cam(assoc loss over 6 positives + 50 hard negatives)

Distribution strategy (class-range column sharding):
  Core k owns classes [k*250, (k+1)*250) of every camera block -- the same
  250-class slice of all 6 cameras, NL=1500 columns per core.  All 8 cores
  run one SPMD program:
    * sims_local = feats @ memT_local  (PE, fp8e4 DoubleRow, scale 16*16;
      PSUM holds 256*sims)
    * per-camera-block partial sum(exp(20*sims))  (ACT Exp accum on PSUM)
    * top-8 of each 250-col camera block          (DVE max8 on PSUM)
  Host merges per-core stats ([256, 54] each): drops positives from the
  candidate lists (host computes exact positive sims), takes the global
  top-50 with an exactness certificate + exact fallback, log-sum-exp
  combines, segment-sums to the scalar loss.

Device schedule: the 6 camera blocks are processed as 3 block-pairs, each
a [128, 512] PSUM tile (2 x 250 real cols padded to 256).  memT is packed
pair-major / ko-major so DMA pieces are per-partition-contiguous; pieces
stream in consumption order round-robin over 3 HWDGE/SWDGE queues, so
matmuls start after ~0.6 MB instead of the whole 3.6 MB.  Per-pair
epilogues (max8 + Exp-accum) overlap the next pair's matmuls; a few
warm-up matmuls on a zero scratch tile hold the PE HAM un-throttled
before the first data lands.  fp8 quantization noise on sims is ~1e-3
(std), ~20x below the 2e-2 loss tolerance budget.
"""

import os
import sys
import types

import numpy as np

# ---------------------------------------------------------------- constants
B = 256          # batch
D = 2048         # feature dim
NCAMS = 6
C = 2000         # classes per camera
NG = NCAMS * C   # 12000 global columns
M = 8            # cores
W = C // M       # 250 classes per core per camera block
P = 128          # partitions
KO = D // P      # 16 contraction subtiles of 128
KP = KO // 2     # 8 DoubleRow ko-pairs
BT = B // P      # 2 batch tiles
NPAIR = 3        # camera-block pairs per core
WB = 256         # padded block width (250 real + 6 zero cols)
WPAIR = 2 * WB   # 512 = one PSUM bank of f32
BETA = 0.05
INV_BETA = 1.0 / BETA        # 20.0
BG_KNN = 50
FSCALE = 16.0                # host pre-scale on feats before fp8 cast
MSCALE = 16.0                # host pre-scale on memory before fp8 cast
PSCALE = FSCALE * MSCALE     # PSUM holds PSCALE * sims
NCAND = NCAMS * 8            # 48 candidates per core (top-8 per block)
GRAN = 16        # outs columns per (bt, block): 8 topk | 1 sumexp | 7 pad
OUTC = NCAMS * GRAN          # 96 outs columns per batch tile
POS_TOL = 8e-3   # host-side positive-candidate matching tolerance (sims units)
N_WARM = 70      # tiny PE warm-up matmuls before real data arrives
N_WARM_IN = 2    # inline warm-ups between pair0 kp groups (fill DMA stalls)
WARM_N = 64      # moving cols per warm-up matmul
BW = B + WPAIR   # 768: bundled feats+pair0 bytes per (partition, ko)

LAST_EXEC_NS = None
FALLBACK_COUNT = 0
_NC_CACHE = {}


def _install_axon_ntff_hook():
    """The agent image's antenv lacks axon_hooks; synthesize it so
    run_bass_kernel_spmd(trace=True) can capture NTFF profiles."""
    if "antenv.axon_hooks" in sys.modules:
        return
    mod = types.ModuleType("antenv.axon_hooks")
    state = {"hook": None}
    mod.set_axon_ntff_profile_hook = lambda h: state.__setitem__("hook", h)
    mod.get_axon_ntff_profile_hook = lambda: state["hook"]
    sys.modules["antenv.axon_hooks"] = mod
    try:
        import antenv

        antenv.axon_hooks = mod
    except Exception:
        pass
    try:
        from trn_agent_boot.trn_boot import _ntff_profile_via_ctypes

        hook = _ntff_profile_via_ctypes("/opt/axon/libaxon_pjrt.so")
        if hook is not None:
            mod.set_axon_ntff_profile_hook(hook)
    except Exception:
        pass


def build_nc():
    """Build + compile the single SPMD Bass program shared by all 8 cores."""
    import concourse.bacc as bacc
    import concourse.mybir as mybir
    import concourse.tile as tile

    f32 = mybir.dt.float32
    fp8 = mybir.dt.float8e4
    AF = mybir.ActivationFunctionType
    DR = mybir.MatmulPerfMode.DoubleRow

    nc = bacc.Bacc(
        "TRN2",
        target_bir_lowering=False,
        debug=False,
        enable_asserts=False,
        num_devices=M,
    )

    # bun: per (partition, ko): [featsT slice (256) | pair0 mem cols (512)],
    # so each kp's whole matmul working set arrives as one DMA piece
    bun_d = nc.dram_tensor("bun", [P, KO * BW], fp8, kind="ExternalInput")
    mem12_d = nc.dram_tensor(
        "mem12", [P, 2 * KO * WPAIR], fp8, kind="ExternalInput"
    )
    out_d = nc.dram_tensor("out", [P, BT * OUTC], f32, kind="ExternalOutput")

    with tile.TileContext(nc) as tc:
        with (
            tc.tile_pool(name="big", bufs=1) as big,
            tc.tile_pool(name="scr", bufs=4) as scr,
            tc.tile_pool(name="psum", bufs=1, space="PSUM") as psum,
        ):
            bun_sb = big.tile([P, KO, BW], fp8)
            mem12_sb = big.tile([P, 2, KO, WPAIR], fp8)
            warm_sb = big.tile([P, WPAIR], fp8)
            outs = big.tile([P, BT * OUTC], f32)

            pstiles = [
                psum.tile([P, WPAIR], f32, tag=f"ps{pr}_{bt}", name=f"ps{pr}_{bt}")
                for pr in range(NPAIR)
                for bt in range(BT)
            ]
            pswarm = psum.tile([P, WPAIR], f32, tag="pswarm")

            # PE warm-up: tiny zero scratch matmuls with no data dependencies
            # keep the HAM activity window busy while the first DMA pieces
            # land (each costs <100ns if data is already there).
            nc.vector.memset(warm_sb[:], 0.0)
            for _ in range(N_WARM):
                nc.tensor.matmul(
                    pswarm[:, :WARM_N],
                    warm_sb[:, 0:P],
                    warm_sb[:, :WARM_N],
                    start=True,
                    stop=True,
                )

            # ---- streaming DMA: pieces in PE-consumption order with explicit
            # queue assignment.  One bundle piece per kp (192 KB) carries that
            # kp's feats AND pair0 columns; pairs 1-2 stream as quarters.
            # gpsimd (q2, SWDGE) starts ~1us slower, so it gets pieces needed
            # later.
            mqueues = [nc.sync, nc.scalar, nc.gpsimd]
            m12v = mem12_d[:].rearrange(
                "p (pr ko w) -> p pr ko w", pr=2, ko=KO, w=WPAIR
            )

            def bun_piece(q, ko):
                mqueues[q].dma_start(
                    bun_sb[:, ko : ko + 1, :],
                    bun_d[:, ko * BW : (ko + 1) * BW],
                )

            def mem_piece(q, pr, klo, khi):
                mqueues[q].dma_start(
                    mem12_sb[:, pr - 1, klo:khi, :], m12v[:, pr - 1, klo:khi, :]
                )

            # need-ordered pieces, 4 per queue (each DMA issue costs the
            # engine ~0.7us, so piece count is itself a budget); early pieces
            # small for latency, later ones big for issue economy
            def bun_range(q, klo, khi):
                mqueues[q].dma_start(
                    bun_sb[:, klo:khi, :], bun_d[:, klo * BW : khi * BW]
                )

            bun_range(0, 0, 2)
            bun_range(1, 2, 4)
            bun_range(2, 4, 6)
            bun_range(0, 6, 9)
            bun_range(1, 9, 12)
            bun_range(2, 12, 16)
            mem_piece(0, 1, 0, 6)
            mem_piece(1, 1, 6, 11)
            mem_piece(2, 1, 11, 16)
            mem_piece(2, 2, 0, 4)
            mem_piece(0, 2, 4, 10)
            mem_piece(1, 2, 10, 16)

            # ---- main pipeline: per (pair, bt): 8 DoubleRow matmuls
            # accumulating ko, then the epilogue on ACT/DVE while the PE
            # moves on to the next group.  Pair 0 walks kp with bt
            # interleaved (two matmuls per arriving ko-piece) since its DMA
            # races the PE; later pairs keep bt sequential so their
            # epilogues stagger.
            def mm(pr, bt, kp):
                rhs = (
                    bun_sb[:, 2 * kp : 2 * kp + 2, B : B + WPAIR]
                    if pr == 0
                    else mem12_sb[:, pr - 1, 2 * kp : 2 * kp + 2, :]
                )
                nc.tensor.matmul(
                    pstiles[pr * BT + bt][:],
                    bun_sb[:, 2 * kp : 2 * kp + 2, bt * P : (bt + 1) * P],
                    rhs,
                    start=(kp == 0),
                    stop=(kp == KP - 1),
                    perf_mode=DR,
                )

            def epilogue(pr, bt, exps_first=False):
                ps = pstiles[pr * BT + bt]

                def maxes():
                    for blk in range(2):
                        base = bt * OUTC + GRAN * (2 * pr + blk)
                        nc.vector.max(
                            out=outs[:, base : base + 8],
                            in_=ps[:, blk * WB : blk * WB + W],
                        )

                def exps():
                    for blk in range(2):
                        j = 2 * pr + blk
                        base = bt * OUTC + GRAN * j
                        et = scr.tile([P, W], fp8, tag="exp", name=f"et{j}_{bt}")
                        nc.scalar.activation(
                            et[:],
                            ps[:, blk * WB : blk * WB + W],
                            AF.Exp,
                            scale=INV_BETA / PSCALE,
                            accum_out=outs[:, base + 8 : base + 9],
                        )

                # the slower ACT chain goes first on the final group so it
                # starts at matmul-done instead of after the DVE max8s
                if exps_first:
                    exps()
                    maxes()
                else:
                    maxes()
                    exps()

            def filler(pr, bt, kp, n):
                # stall-filler pinned in schedule position: it reads the
                # CURRENT kp's already-required data (so the Tile scheduler
                # cannot hoist it to the front the way a dependency-free
                # matmul gets hoisted), reuses the preceding matmul's
                # stationary operand (no fresh LDWEIGHTS), writes the scratch
                # PSUM bank, and keeps the PE HAM clock-gate window busy
                # across DMA arrival jitter.  ~70ns each when data is on
                # time.
                rhs = (
                    bun_sb[:, 2 * kp : 2 * kp + 2, B : B + WARM_N]
                    if pr == 0
                    else mem12_sb[:, pr - 1, 2 * kp : 2 * kp + 2, :WARM_N]
                )
                for _ in range(n):
                    nc.tensor.matmul(
                        pswarm[:, :WARM_N],
                        bun_sb[:, 2 * kp : 2 * kp + 2, bt * P : (bt + 1) * P],
                        rhs,
                        start=True,
                        stop=True,
                        perf_mode=DR,
                    )

            for kp in range(KP):
                for bt in range(BT):
                    mm(0, bt, kp)
                filler(0, 1, kp, N_WARM_IN)
            epilogue(0, 0)
            epilogue(0, 1)
            for pr in range(1, NPAIR):
                for bt in range(BT):
                    for kp in range(KP):
                        mm(pr, bt, kp)
                        filler(pr, bt, kp, 1)
                    epilogue(pr, bt)

            # split output DMA: bt0's half issues while bt1's last epilogue
            # still runs, overlapping most of the first store's flight
            nc.sync.dma_start(out_d[:, :OUTC], outs[:, :OUTC])
            nc.scalar.dma_start(out_d[:, OUTC:], outs[:, OUTC:])

    nc.compile()
    return nc


def get_nc():
    if "nc" not in _NC_CACHE:
        _NC_CACHE["nc"] = build_nc()
    return _NC_CACHE["nc"]


def _fp8():
    import ml_dtypes

    return np.dtype(ml_dtypes.float8_e4m3fn)


def shard_classes(k: int) -> np.ndarray:
    """Global memory-bank columns owned by core k (same class range of
    every camera block)."""
    return (
        np.arange(NCAMS)[:, None] * C + k * W + np.arange(W)[None, :]
    ).reshape(-1)


def pack_featsT(features: np.ndarray) -> np.ndarray:
    """[B, D] -> [P, KO, B] fp8, row p holding feats.T[ko*128+p, :] runs."""
    arr = (features * FSCALE).astype(_fp8())
    return np.ascontiguousarray(arr.T.reshape(KO, P, B).transpose(1, 0, 2))


def pack_memT(mem8_core: np.ndarray) -> np.ndarray:
    """[6, 250, D] fp8 -> [P, NPAIR, KO, WPAIR] in (pair, ko, blk, col)
    order with each 250-col block zero-padded to 256."""
    Xp = np.zeros((NCAMS, WB, D), dtype=mem8_core.dtype)
    Xp[:, :W, :] = mem8_core
    # [pair, blk, c, ko, p] -> [p, pair, ko, blk, c]
    return Xp.reshape(NPAIR, 2, WB, KO, P).transpose(4, 0, 3, 1, 2)


def pack_inputs(featsT: np.ndarray, Y: np.ndarray):
    """featsT [P, KO, B] + Y [P, NPAIR, KO, 2, WB] -> (bun [P, KO*BW],
    mem12 [P, 2*KO*WPAIR]) device arrays."""
    bun = np.empty((P, KO, BW), dtype=featsT.dtype)
    bun[:, :, :B] = featsT
    bun[:, :, B:] = Y[:, 0].reshape(P, KO, WPAIR)
    mem12 = Y[:, 1:].reshape(P, 2 * KO * WPAIR)
    return (
        np.ascontiguousarray(bun.reshape(P, KO * BW)),
        np.ascontiguousarray(mem12),
    )


def _loss_from_parts(pos_logits, lse_block, top50, cams):
    rows = np.arange(B)
    ce = lse_block[rows, cams] - pos_logits[rows, cams]
    logits = np.concatenate([pos_logits, INV_BETA * top50], axis=1)
    mx = logits.max(axis=1, keepdims=True)
    lse56 = mx[:, 0] + np.log(np.exp(logits - mx).sum(axis=1))
    assoc = lse56 - pos_logits.sum(axis=1) / NCAMS

    counts = np.bincount(cams, minlength=NCAMS).astype(np.float64)
    ce_sum = np.bincount(cams, weights=ce, minlength=NCAMS)
    as_sum = np.bincount(cams, weights=assoc, minlength=NCAMS)
    safe = np.maximum(counts, 1.0)
    present = counts > 0
    return np.sum(np.where(present, ce_sum / safe, 0.0)) + np.sum(
        np.where(present, 0.5 * as_sum / safe, 0.0)
    )


def host_combine(outs, features, memory, cams, labels):
    """outs: [M, B, OUTC] device results (candidates scaled by PSCALE);
    per camera block j: cols [16j:16j+8] top-8, col 16j+8 sum-exp."""
    global FALLBACK_COUNT
    g = outs.reshape(M, B, NCAMS, GRAN).astype(np.float64)
    cand = (g[:, :, :, :8] / PSCALE).reshape(M, B, NCAND)  # [M, B, 48]
    sexp = g[:, :, :, 8]                                   # [M, B, 6]

    s_block = sexp.sum(axis=0)   # [B, 6] sum(exp(20*sims)) per camera block
    lse_block = np.log(s_block)  # logsumexp of own-camera logits

    # positives: one dot product per (row, camera) -- 6.3 MFLOP on host
    feats64 = np.asarray(features, np.float64)
    pos_vals = np.einsum(
        "bd,jbd->bj",
        feats64,
        np.asarray(memory, np.float64)[:, labels, :],
        optimize=True,
    )  # [B, 6]

    # [B, M*6, 8] per-(core, camera-block) candidate lists
    percl = cand.transpose(1, 0, 2).reshape(B, M * NCAMS, 8).copy()
    cmin_raw = percl.min(axis=2)  # pre-drop floor per (core, block)

    # Remove positives from the candidate lists.  Positive (i, j) can only
    # appear on core labels[i]//W in block j; drop the closest value within
    # POS_TOL (missing a true positive corrupts the hard negatives; an
    # over-drop of a near-equal genuine value is harmless).
    own_core = labels // W  # [B]
    rows = np.arange(B)
    for j in range(NCAMS):
        cl = own_core * NCAMS + j  # [B] candidate-list index
        lists = percl[rows, cl]    # [B, 8] (fancy-index copy)
        diff = np.abs(lists - pos_vals[:, j : j + 1])
        am = diff.argmin(axis=1)
        hit = diff[rows, am] < POS_TOL
        lists[hit, am[hit]] = -np.inf
        percl[rows, cl] = lists

    flat = percl.reshape(B, -1)
    top50 = -np.partition(-flat, BG_KNN - 1, axis=1)[:, :BG_KNN]
    t50 = top50[:, BG_KNN - 1]  # [B] 50th largest of the union

    # Exactness certificate: every (core, block)'s smallest extracted
    # candidate must lie strictly below the union's 50th value, proving no
    # unseen value could reach the global top-50.
    bad = (cmin_raw >= t50[:, None]).any(axis=1)
    if bad.any():
        # Exact fallback for insufficient rows: recompute on the host.
        FALLBACK_COUNT += int(bad.sum())
        mem_flat = np.asarray(memory, np.float32).reshape(NG, D)
        idx = np.nonzero(bad)[0]
        sims = np.asarray(features, np.float32)[idx] @ mem_flat.T
        colsg = np.arange(NG)
        for p, i in enumerate(idx):
            row = sims[p].astype(np.float64)
            row[colsg % C == labels[i]] = -np.inf
            top50[i] = -np.sort(-row)[:BG_KNN]

    return np.float32(
        _loss_from_parts(INV_BETA * pos_vals, lse_block, top50, cams)
    )


def kernel(features, memory, cams, labels, trace: bool = None):
    global LAST_EXEC_NS
    _install_axon_ntff_hook()
    from concourse.bass_utils import run_bass_kernel_spmd

    features = np.asarray(features, dtype=np.float32)
    memory = np.asarray(memory, dtype=np.float32)
    cams = np.asarray(cams).astype(np.int64)
    labels = np.asarray(labels).astype(np.int64)

    nc = get_nc()

    mem_flat = memory.reshape(NG, D)
    mem8 = np.clip(mem_flat * MSCALE, -240.0, 240.0).astype(_fp8())
    featsT = pack_featsT(features)
    in_maps = []
    for k in range(M):
        Y = pack_memT(mem8[shard_classes(k)].reshape(NCAMS, W, D))
        bun, mem12 = pack_inputs(featsT, Y)
        in_maps.append({"bun": bun, "mem12": mem12})

    if trace is None:
        trace = os.environ.get("CAP_TRACE", "1") == "1"
    res = run_bass_kernel_spmd(
        nc, in_maps, core_ids=list(range(M)), trace=trace
    )
    if res.exec_time_ns is not None:
        LAST_EXEC_NS = res.exec_time_ns

    outs = np.stack(
        [
            np.concatenate(
                [r["out"][:, :OUTC], r["out"][:, OUTC:]], axis=0
            )
            for r in res.results
        ]
    )  # [M, B, OUTC]
    return np.asarray(
        host_combine(outs, features, memory, cams, labels), dtype=np.float32
    )


# ------------------------------------------------------------------ helpers
def expected_core_out(features, memory, labels, k: int) -> np.ndarray:
    """Numpy model of what core k's device program should output [B, OUTC]
    (modulo fp8 quantization)."""
    mem_flat = np.asarray(memory, np.float32).reshape(NG, D)
    cols = shard_classes(k)
    sims = np.asarray(features, np.float32) @ mem_flat[cols].T  # [B, 1500]
    out = np.zeros((B, OUTC), np.float32)
    for j in range(NCAMS):
        jsl = slice(j * W, (j + 1) * W)
        out[:, GRAN * j + 8] = np.exp(
            INV_BETA * sims[:, jsl].astype(np.float64)
        ).sum(axis=1)
        srt = -np.sort(-sims[:, jsl], axis=1)
        out[:, GRAN * j : GRAN * j + 8] = PSCALE * srt[:, :8]
    return out


# revision 36
# speedup vs baseline: 1.0328x; 1.0328x over previous
"""Distributed CAP-memory loss kernel for 8 TRN2 NeuronCores (fp8 v2).

Problem (see reference): given unit-norm features [B=256, D=2048] and a
memory bank [6, 2000, 2048], compute
  loss = sum_cam mean_cam(per-camera proxy CE)
       + 0.5 * sum_cam mean_cam(assoc loss over 6 positives + 50 hard negatives)

Distribution strategy (class-range column sharding):
  Core k owns classes [k*250, (k+1)*250) of every camera block -- the same
  250-class slice of all 6 cameras, NL=1500 columns per core.  All 8 cores
  run one SPMD program:
    * sims_local = feats @ memT_local  (PE, fp8e4 DoubleRow, scale 16*16;
      PSUM holds 256*sims)
    * per-camera-block partial sum(exp(20*sims))  (ACT Exp accum on PSUM)
    * top-8 of each 250-col camera block          (DVE max8 on PSUM)
  Host merges per-core stats ([256, 54] each): drops positives from the
  candidate lists (host computes exact positive sims), takes the global
  top-50 with an exactness certificate + exact fallback, log-sum-exp
  combines, segment-sums to the scalar loss.

Device schedule: the 6 camera blocks are processed as 3 block-pairs, each
a [128, 512] PSUM tile (2 x 250 real cols padded to 256).  memT is packed
pair-major / ko-major so DMA pieces are per-partition-contiguous; pieces
stream in consumption order round-robin over 3 HWDGE/SWDGE queues, so
matmuls start after ~0.6 MB instead of the whole 3.6 MB.  Per-pair
epilogues (max8 + Exp-accum) overlap the next pair's matmuls; a few
warm-up matmuls on a zero scratch tile hold the PE HAM un-throttled
before the first data lands.  fp8 quantization noise on sims is ~1e-3
(std), ~20x below the 2e-2 loss tolerance budget.
"""

import os
import sys
import types

import numpy as np

# ---------------------------------------------------------------- constants
B = 256          # batch
D = 2048         # feature dim
NCAMS = 6
C = 2000         # classes per camera
NG = NCAMS * C   # 12000 global columns
M = 8            # cores
W = C // M       # 250 classes per core per camera block
P = 128          # partitions
KO = D // P      # 16 contraction subtiles of 128
KP = KO // 2     # 8 DoubleRow ko-pairs
BT = B // P      # 2 batch tiles
NPAIR = 3        # camera-block pairs per core
WB = 256         # padded block width (250 real + 6 zero cols)
WPAIR = 2 * WB   # 512 = one PSUM bank of f32
BETA = 0.05
INV_BETA = 1.0 / BETA        # 20.0
BG_KNN = 50
FSCALE = 16.0                # host pre-scale on feats before fp8 cast
MSCALE = 16.0                # host pre-scale on memory before fp8 cast
PSCALE = FSCALE * MSCALE     # PSUM holds PSCALE * sims
NCAND = NCAMS * 8            # 48 candidates per core (top-8 per block)
GRAN = 16        # outs columns per (bt, block): 8 topk | 1 sumexp | 7 pad
OUTC = NCAMS * GRAN          # 96 outs columns per batch tile
POS_TOL = 8e-3   # host-side positive-candidate matching tolerance (sims units)
N_WARM = 70      # tiny PE warm-up matmuls before real data arrives
N_WARM_IN = 2    # inline warm-ups between pair0 kp groups (fill DMA stalls)
WARM_N = 64      # moving cols per warm-up matmul
BW = B + WPAIR   # 768: bundled feats+pair0 bytes per (partition, ko)

LAST_EXEC_NS = None
FALLBACK_COUNT = 0
_NC_CACHE = {}


def _install_axon_ntff_hook():
    """The agent image's antenv lacks axon_hooks; synthesize it so
    run_bass_kernel_spmd(trace=True) can capture NTFF profiles."""
    if "antenv.axon_hooks" in sys.modules:
        return
    mod = types.ModuleType("antenv.axon_hooks")
    state = {"hook": None}
    mod.set_axon_ntff_profile_hook = lambda h: state.__setitem__("hook", h)
    mod.get_axon_ntff_profile_hook = lambda: state["hook"]
    sys.modules["antenv.axon_hooks"] = mod
    try:
        import antenv

        antenv.axon_hooks = mod
    except Exception:
        pass
    try:
        from trn_agent_boot.trn_boot import _ntff_profile_via_ctypes

        hook = _ntff_profile_via_ctypes("/opt/axon/libaxon_pjrt.so")
        if hook is not None:
            mod.set_axon_ntff_profile_hook(hook)
    except Exception:
        pass


def build_nc():
    """Build + compile the single SPMD Bass program shared by all 8 cores."""
    import concourse.bacc as bacc
    import concourse.mybir as mybir
    import concourse.tile as tile

    f32 = mybir.dt.float32
    fp8 = mybir.dt.float8e4
    AF = mybir.ActivationFunctionType
    DR = mybir.MatmulPerfMode.DoubleRow

    nc = bacc.Bacc(
        "TRN2",
        target_bir_lowering=False,
        debug=False,
        enable_asserts=False,
        num_devices=M,
    )

    # bun: per (partition, ko): [featsT slice (256) | pair0 mem cols (512)],
    # so each kp's whole matmul working set arrives as one DMA piece
    bun_d = nc.dram_tensor("bun", [P, KO * BW], fp8, kind="ExternalInput")
    mem12_d = nc.dram_tensor(
        "mem12", [P, 2 * KO * WPAIR], fp8, kind="ExternalInput"
    )
    out_d = nc.dram_tensor("out", [P, BT * OUTC], f32, kind="ExternalOutput")

    with tile.TileContext(nc) as tc:
        with (
            tc.tile_pool(name="big", bufs=1) as big,
            tc.tile_pool(name="scr", bufs=4) as scr,
            tc.tile_pool(name="psum", bufs=1, space="PSUM") as psum,
        ):
            bun_sb = big.tile([P, KO, BW], fp8)
            mem12_sb = big.tile([P, 2, KO, WPAIR], fp8)
            warm_sb = big.tile([P, WPAIR], fp8)
            outs = big.tile([P, BT * OUTC], f32)

            pstiles = [
                psum.tile([P, WPAIR], f32, tag=f"ps{pr}_{bt}", name=f"ps{pr}_{bt}")
                for pr in range(NPAIR)
                for bt in range(BT)
            ]
            pswarm = psum.tile([P, WPAIR], f32, tag="pswarm")

            # PE warm-up: tiny zero scratch matmuls with no data dependencies
            # keep the HAM activity window busy while the first DMA pieces
            # land (each costs <100ns if data is already there).
            nc.vector.memset(warm_sb[:], 0.0)
            for _ in range(N_WARM):
                nc.tensor.matmul(
                    pswarm[:, :WARM_N],
                    warm_sb[:, 0:P],
                    warm_sb[:, :WARM_N],
                    start=True,
                    stop=True,
                )

            # ---- streaming DMA: pieces in PE-consumption order with explicit
            # queue assignment.  One bundle piece per kp (192 KB) carries that
            # kp's feats AND pair0 columns; pairs 1-2 stream as quarters.
            # gpsimd (q2, SWDGE) starts ~1us slower, so it gets pieces needed
            # later.
            mqueues = [nc.sync, nc.scalar, nc.gpsimd]
            m12v = mem12_d[:].rearrange(
                "p (pr ko w) -> p pr ko w", pr=2, ko=KO, w=WPAIR
            )

            def bun_piece(q, ko):
                mqueues[q].dma_start(
                    bun_sb[:, ko : ko + 1, :],
                    bun_d[:, ko * BW : (ko + 1) * BW],
                )

            def mem_piece(q, pr, klo, khi):
                mqueues[q].dma_start(
                    mem12_sb[:, pr - 1, klo:khi, :], m12v[:, pr - 1, klo:khi, :]
                )

            # need-ordered pieces, 4 per queue (each DMA issue costs the
            # engine ~0.7us, so piece count is itself a budget); early pieces
            # small for latency, later ones big for issue economy
            def bun_range(q, klo, khi):
                mqueues[q].dma_start(
                    bun_sb[:, klo:khi, :], bun_d[:, klo * BW : khi * BW]
                )

            bun_range(0, 0, 2)
            bun_range(1, 2, 4)
            bun_range(0, 4, 7)
            bun_range(1, 7, 10)
            bun_range(0, 10, 13)
            bun_range(1, 13, 16)
            mem_piece(0, 1, 0, 8)
            mem_piece(1, 1, 8, 16)
            mem_piece(0, 2, 0, 8)
            mem_piece(1, 2, 8, 16)

            # ---- main pipeline: per (pair, bt): 8 DoubleRow matmuls
            # accumulating ko, then the epilogue on ACT/DVE while the PE
            # moves on to the next group.  Pair 0 walks kp with bt
            # interleaved (two matmuls per arriving ko-piece) since its DMA
            # races the PE; later pairs keep bt sequential so their
            # epilogues stagger.
            def mm(pr, bt, kp):
                rhs = (
                    bun_sb[:, 2 * kp : 2 * kp + 2, B : B + WPAIR]
                    if pr == 0
                    else mem12_sb[:, pr - 1, 2 * kp : 2 * kp + 2, :]
                )
                nc.tensor.matmul(
                    pstiles[pr * BT + bt][:],
                    bun_sb[:, 2 * kp : 2 * kp + 2, bt * P : (bt + 1) * P],
                    rhs,
                    start=(kp == 0),
                    stop=(kp == KP - 1),
                    perf_mode=DR,
                )

            def epilogue(pr, bt, exps_first=False):
                ps = pstiles[pr * BT + bt]

                def maxes():
                    for blk in range(2):
                        base = bt * OUTC + GRAN * (2 * pr + blk)
                        nc.vector.max(
                            out=outs[:, base : base + 8],
                            in_=ps[:, blk * WB : blk * WB + W],
                        )

                def exps():
                    for blk in range(2):
                        j = 2 * pr + blk
                        base = bt * OUTC + GRAN * j
                        et = scr.tile([P, W], fp8, tag="exp", name=f"et{j}_{bt}")
                        nc.scalar.activation(
                            et[:],
                            ps[:, blk * WB : blk * WB + W],
                            AF.Exp,
                            scale=INV_BETA / PSCALE,
                            accum_out=outs[:, base + 8 : base + 9],
                        )

                # the slower ACT chain goes first on the final group so it
                # starts at matmul-done instead of after the DVE max8s
                if exps_first:
                    exps()
                    maxes()
                else:
                    maxes()
                    exps()

            def filler(pr, bt, kp, n):
                # stall-filler pinned in schedule position: it reads the
                # CURRENT kp's already-required data (so the Tile scheduler
                # cannot hoist it to the front the way a dependency-free
                # matmul gets hoisted), reuses the preceding matmul's
                # stationary operand (no fresh LDWEIGHTS), writes the scratch
                # PSUM bank, and keeps the PE HAM clock-gate window busy
                # across DMA arrival jitter.  ~70ns each when data is on
                # time.
                rhs = (
                    bun_sb[:, 2 * kp : 2 * kp + 2, B : B + WARM_N]
                    if pr == 0
                    else mem12_sb[:, pr - 1, 2 * kp : 2 * kp + 2, :WARM_N]
                )
                for _ in range(n):
                    nc.tensor.matmul(
                        pswarm[:, :WARM_N],
                        bun_sb[:, 2 * kp : 2 * kp + 2, bt * P : (bt + 1) * P],
                        rhs,
                        start=True,
                        stop=True,
                        perf_mode=DR,
                    )

            for kp in range(KP):
                for bt in range(BT):
                    mm(0, bt, kp)
                filler(0, 1, kp, N_WARM_IN)
            epilogue(0, 0)
            epilogue(0, 1)
            for pr in range(1, NPAIR):
                for bt in range(BT):
                    for kp in range(KP):
                        mm(pr, bt, kp)
                        filler(pr, bt, kp, 1)
                    epilogue(pr, bt)

            # split output DMA: bt0's half issues while bt1's last epilogue
            # still runs, overlapping most of the first store's flight
            nc.sync.dma_start(out_d[:, :OUTC], outs[:, :OUTC])
            nc.scalar.dma_start(out_d[:, OUTC:], outs[:, OUTC:])

    nc.compile()
    return nc


def get_nc():
    if "nc" not in _NC_CACHE:
        _NC_CACHE["nc"] = build_nc()
    return _NC_CACHE["nc"]


def _fp8():
    import ml_dtypes

    return np.dtype(ml_dtypes.float8_e4m3fn)


def shard_classes(k: int) -> np.ndarray:
    """Global memory-bank columns owned by core k (same class range of
    every camera block)."""
    return (
        np.arange(NCAMS)[:, None] * C + k * W + np.arange(W)[None, :]
    ).reshape(-1)


def pack_featsT(features: np.ndarray) -> np.ndarray:
    """[B, D] -> [P, KO, B] fp8, row p holding feats.T[ko*128+p, :] runs."""
    arr = (features * FSCALE).astype(_fp8())
    return np.ascontiguousarray(arr.T.reshape(KO, P, B).transpose(1, 0, 2))


def pack_memT(mem8_core: np.ndarray) -> np.ndarray:
    """[6, 250, D] fp8 -> [P, NPAIR, KO, WPAIR] in (pair, ko, blk, col)
    order with each 250-col block zero-padded to 256."""
    Xp = np.zeros((NCAMS, WB, D), dtype=mem8_core.dtype)
    Xp[:, :W, :] = mem8_core
    # [pair, blk, c, ko, p] -> [p, pair, ko, blk, c]
    return Xp.reshape(NPAIR, 2, WB, KO, P).transpose(4, 0, 3, 1, 2)


def pack_inputs(featsT: np.ndarray, Y: np.ndarray):
    """featsT [P, KO, B] + Y [P, NPAIR, KO, 2, WB] -> (bun [P, KO*BW],
    mem12 [P, 2*KO*WPAIR]) device arrays."""
    bun = np.empty((P, KO, BW), dtype=featsT.dtype)
    bun[:, :, :B] = featsT
    bun[:, :, B:] = Y[:, 0].reshape(P, KO, WPAIR)
    mem12 = Y[:, 1:].reshape(P, 2 * KO * WPAIR)
    return (
        np.ascontiguousarray(bun.reshape(P, KO * BW)),
        np.ascontiguousarray(mem12),
    )


def _loss_from_parts(pos_logits, lse_block, top50, cams):
    rows = np.arange(B)
    ce = lse_block[rows, cams] - pos_logits[rows, cams]
    logits = np.concatenate([pos_logits, INV_BETA * top50], axis=1)
    mx = logits.max(axis=1, keepdims=True)
    lse56 = mx[:, 0] + np.log(np.exp(logits - mx).sum(axis=1))
    assoc = lse56 - pos_logits.sum(axis=1) / NCAMS

    counts = np.bincount(cams, minlength=NCAMS).astype(np.float64)
    ce_sum = np.bincount(cams, weights=ce, minlength=NCAMS)
    as_sum = np.bincount(cams, weights=assoc, minlength=NCAMS)
    safe = np.maximum(counts, 1.0)
    present = counts > 0
    return np.sum(np.where(present, ce_sum / safe, 0.0)) + np.sum(
        np.where(present, 0.5 * as_sum / safe, 0.0)
    )


def host_combine(outs, features, memory, cams, labels):
    """outs: [M, B, OUTC] device results (candidates scaled by PSCALE);
    per camera block j: cols [16j:16j+8] top-8, col 16j+8 sum-exp."""
    global FALLBACK_COUNT
    g = outs.reshape(M, B, NCAMS, GRAN).astype(np.float64)
    cand = (g[:, :, :, :8] / PSCALE).reshape(M, B, NCAND)  # [M, B, 48]
    sexp = g[:, :, :, 8]                                   # [M, B, 6]

    s_block = sexp.sum(axis=0)   # [B, 6] sum(exp(20*sims)) per camera block
    lse_block = np.log(s_block)  # logsumexp of own-camera logits

    # positives: one dot product per (row, camera) -- 6.3 MFLOP on host
    feats64 = np.asarray(features, np.float64)
    pos_vals = np.einsum(
        "bd,jbd->bj",
        feats64,
        np.asarray(memory, np.float64)[:, labels, :],
        optimize=True,
    )  # [B, 6]

    # [B, M*6, 8] per-(core, camera-block) candidate lists
    percl = cand.transpose(1, 0, 2).reshape(B, M * NCAMS, 8).copy()
    cmin_raw = percl.min(axis=2)  # pre-drop floor per (core, block)

    # Remove positives from the candidate lists.  Positive (i, j) can only
    # appear on core labels[i]//W in block j; drop the closest value within
    # POS_TOL (missing a true positive corrupts the hard negatives; an
    # over-drop of a near-equal genuine value is harmless).
    own_core = labels // W  # [B]
    rows = np.arange(B)
    for j in range(NCAMS):
        cl = own_core * NCAMS + j  # [B] candidate-list index
        lists = percl[rows, cl]    # [B, 8] (fancy-index copy)
        diff = np.abs(lists - pos_vals[:, j : j + 1])
        am = diff.argmin(axis=1)
        hit = diff[rows, am] < POS_TOL
        lists[hit, am[hit]] = -np.inf
        percl[rows, cl] = lists

    flat = percl.reshape(B, -1)
    top50 = -np.partition(-flat, BG_KNN - 1, axis=1)[:, :BG_KNN]
    t50 = top50[:, BG_KNN - 1]  # [B] 50th largest of the union

    # Exactness certificate: every (core, block)'s smallest extracted
    # candidate must lie strictly below the union's 50th value, proving no
    # unseen value could reach the global top-50.
    bad = (cmin_raw >= t50[:, None]).any(axis=1)
    if bad.any():
        # Exact fallback for insufficient rows: recompute on the host.
        FALLBACK_COUNT += int(bad.sum())
        mem_flat = np.asarray(memory, np.float32).reshape(NG, D)
        idx = np.nonzero(bad)[0]
        sims = np.asarray(features, np.float32)[idx] @ mem_flat.T
        colsg = np.arange(NG)
        for p, i in enumerate(idx):
            row = sims[p].astype(np.float64)
            row[colsg % C == labels[i]] = -np.inf
            top50[i] = -np.sort(-row)[:BG_KNN]

    return np.float32(
        _loss_from_parts(INV_BETA * pos_vals, lse_block, top50, cams)
    )


def kernel(features, memory, cams, labels, trace: bool = None):
    global LAST_EXEC_NS
    _install_axon_ntff_hook()
    from concourse.bass_utils import run_bass_kernel_spmd

    features = np.asarray(features, dtype=np.float32)
    memory = np.asarray(memory, dtype=np.float32)
    cams = np.asarray(cams).astype(np.int64)
    labels = np.asarray(labels).astype(np.int64)

    nc = get_nc()

    mem_flat = memory.reshape(NG, D)
    mem8 = np.clip(mem_flat * MSCALE, -240.0, 240.0).astype(_fp8())
    featsT = pack_featsT(features)
    in_maps = []
    for k in range(M):
        Y = pack_memT(mem8[shard_classes(k)].reshape(NCAMS, W, D))
        bun, mem12 = pack_inputs(featsT, Y)
        in_maps.append({"bun": bun, "mem12": mem12})

    if trace is None:
        trace = os.environ.get("CAP_TRACE", "1") == "1"
    res = run_bass_kernel_spmd(
        nc, in_maps, core_ids=list(range(M)), trace=trace
    )
    if res.exec_time_ns is not None:
        LAST_EXEC_NS = res.exec_time_ns

    outs = np.stack(
        [
            np.concatenate(
                [r["out"][:, :OUTC], r["out"][:, OUTC:]], axis=0
            )
            for r in res.results
        ]
    )  # [M, B, OUTC]
    return np.asarray(
        host_combine(outs, features, memory, cams, labels), dtype=np.float32
    )


# ------------------------------------------------------------------ helpers
def expected_core_out(features, memory, labels, k: int) -> np.ndarray:
    """Numpy model of what core k's device program should output [B, OUTC]
    (modulo fp8 quantization)."""
    mem_flat = np.asarray(memory, np.float32).reshape(NG, D)
    cols = shard_classes(k)
    sims = np.asarray(features, np.float32) @ mem_flat[cols].T  # [B, 1500]
    out = np.zeros((B, OUTC), np.float32)
    for j in range(NCAMS):
        jsl = slice(j * W, (j + 1) * W)
        out[:, GRAN * j + 8] = np.exp(
            INV_BETA * sims[:, jsl].astype(np.float64)
        ).sum(axis=1)
        srt = -np.sort(-sims[:, jsl], axis=1)
        out[:, GRAN * j : GRAN * j + 8] = PSCALE * srt[:, :8]
    return out
